# Initial kernel scaffold
#
"""Causal multi-head self-attention (RoPE) Trainium2 Bass kernel (v2).

Problem: x:(4,2048,1024), Wq/Wk/Wv:(1024,1024), Wo:(1024,1024), bo:(1024,)
  q,k,v = split_heads(x@W*), rope(q), rope(k), causal softmax(q k^T/8) v, @Wo+bo

Sharding: head-parallel across 8 cores. Core c owns heads {2c, 2c+1} for all
4 batches: it computes q/k/v projections against the 128-column weight slice,
attention for its heads, and a partial output projection against the matching
128-row slice of Wo. Host sums the 8 partial (8192,1024) fp16 outputs and
adds bo.

v2 schedule: a flat software pipeline. The attention j-loop of batch b is the
clock; each step emits (on PE, in order) the st pair two steps ahead, pumped
projection/rope/V-transpose units of batch b+1, then the AV pair for step j.
EXP runs one step ahead of its AV consumer so the PE never waits on ACT.
Normalize + output-projection work is deferred through a slotted chain so no
engine FIFO ever parks behind the DRAM broadcast round trip.

On-core layout (all "T" tensors feature-major: partitions=feature, free=tok):
  Q^T/K^T (128 x 2048/batch): rows = [h0 d-evens(32), h0 d-odds(32), h1 ...]
  RoPE: dst = qr*cos - P2@(qr*sin2); P2 is block-diagonal per head, so the
    rot matmul runs as two concurrent 64x64 col-tiled matmuls.
  S^T tiles (tj x ti) = K^T.T @ Q^T per head (64-contraction, the two heads
    run concurrently via row tiling). A = exp(0.125*S^T) in fp16 (max logit
    ~3.6 -> exp <= ~40, far from fp16 overflow); diagonal straddle tiles are
    masked AFTER exp by a 0/1 fp16 triangle multiply (2x DVE mode).
  O~^T (65 x ti) accumulated = [V|1].T @ A over tj chunks; row 64 = softmax
    denominators. Normalize via DRAM-staged reciprocal broadcast + DVE mult.
  y partial (128t x 1024) = O^T-chunk.T @ Wo-slice, fp16, psum->sbuf->DRAM.
"""

import numpy as np

B, T, C = 4, 2048, 1024
H, D = 16, 64
N_CORES = 8
BT = B * T
SCALE = 0.125  # D**-0.5

TRACE = False            # set True (e.g. from test.py) to capture an NTFF trace
LAST_RESULT = None       # BassKernelResults of the most recent run

_BUILT = None            # cached nc


# --------------------------------------------------------------------------
# workaround: this walrus build rejects >1 semaphore wait per instruction
def _split_sem_waits(nc, max_waits=1):
    import concourse.mybir as mybir

    n = 0
    for f in nc.m.functions:
        for bb in f.blocks:
            insts = bb.instructions
            idx = 0
            while idx < len(insts):
                i = insts[idx]
                si = getattr(i, "sync_info", None)
                if si is not None and si.on_wait and len(si.on_wait) > max_waits:
                    waits = list(si.on_wait)
                    extra, keep = waits[:-max_waits], waits[-max_waits:]
                    si.on_wait = keep
                    pos = idx
                    for j in range(0, len(extra), max_waits):
                        n += 1
                        nd = mybir.InstNoOp(name=f"I-waitsplit-{n}", ins=[], outs=[])
                        nd.engine = i.engine
                        nd.sync_info = mybir.SyncInfo(
                            on_wait=extra[j : j + max_waits], on_update=[]
                        )
                        insts.insert(pos, nd)
                        pos += 1
                    idx = pos
                idx += 1


def _install_ntff_hook():
    """The image's antenv lacks axon_hooks; synthesize it so trace=True works."""
    import sys
    import types

    if "antenv.axon_hooks" in sys.modules:
        return
    import antenv

    state = {"hook": None}
    mod = types.ModuleType("antenv.axon_hooks")
    mod.get_axon_ntff_profile_hook = lambda: state["hook"]
    mod.set_axon_ntff_profile_hook = lambda h: state.__setitem__("hook", h)
    sys.modules["antenv.axon_hooks"] = mod
    antenv.axon_hooks = mod
    try:
        import contextlib

        from trn_agent_boot.trn_boot import _ntff_profile_via_ctypes

        inner = _ntff_profile_via_ctypes("/opt/axon/libaxon_pjrt.so")

        # axon_start_nrt_profile needs the PJRT client fully initialized;
        # retry with a forced execute between attempts, degrade to
        # no-profile rather than crash the whole run
        @contextlib.contextmanager
        def robust_hook(output_dir, device_ids):
            import jax.numpy as jnp

            ctx = None
            for attempt in range(3):
                try:
                    jnp.zeros(8).block_until_ready()
                    c = inner(output_dir, device_ids)
                    c.__enter__()
                    ctx = c
                    break
                except Exception as e:  # profile start failed; retry
                    print(f"NTFF start attempt {attempt} failed: {e}")
                    import time as _t

                    _t.sleep(1.0)
            if ctx is None:
                print("NTFF profile unavailable; running without trace")
            try:
                yield
            finally:
                if ctx is not None:
                    ctx.__exit__(None, None, None)

        state["hook"] = robust_hook if inner is not None else None
    except Exception:
        state["hook"] = None


# --------------------------------------------------------------------------
def _build():
    import concourse.bass as bass
    import concourse.mybir as mybir
    from concourse.tile import TileContext

    F = mybir.dt.float32
    MD = mybir.dt.float16  # matmul operand dtype
    MULT = mybir.AluOpType.mult
    SUB = mybir.AluOpType.subtract
    EXP = mybir.ActivationFunctionType.Exp

    nc = bass.Bass()

    xT = nc.dram_tensor("xT", (C, BT), MD, kind="ExternalInput")
    wq = nc.dram_tensor("wq", (C, 128), MD, kind="ExternalInput")
    wk = nc.dram_tensor("wk", (C, 128), MD, kind="ExternalInput")
    wv = nc.dram_tensor("wv", (C, 128), MD, kind="ExternalInput")
    wo = nc.dram_tensor("wo", (128, C), MD, kind="ExternalInput")
    cosd = nc.dram_tensor("cos", (128, T), MD, kind="ExternalInput")
    sind = nc.dram_tensor("sin2", (128, T), MD, kind="ExternalInput")
    p2d = nc.dram_tensor("p2", (128, 128), MD, kind="ExternalInput")
    trid = nc.dram_tensor("tri2x", (128, 256), MD, kind="ExternalInput")
    id2d = nc.dram_tensor("id2", (128, 64), F, kind="ExternalInput")
    y = nc.dram_tensor("y", (BT, C), MD, kind="ExternalOutput")
    scr = nc.dram_tensor("scr", (B * 8, 512), F, kind="Internal")

    with TileContext(nc) as tc:
        with (
            tc.tile_pool(name="const", bufs=1) as cst,
            tc.tile_pool(name="xt", bufs=3) as xtp,
            tc.tile_pool(name="qt", bufs=2) as qp,
            tc.tile_pool(name="kt", bufs=2) as kp,
            tc.tile_pool(name="ot", bufs=2) as op_,
            tc.tile_pool(name="vst", bufs=2) as vstp,
            tc.tile_pool(name="tmp", bufs=4) as tmp,
            tc.tile_pool(name="at", bufs=6) as ap_,
            tc.tile_pool(name="bc", bufs=4) as bcp,
            tc.tile_pool(name="avs", bufs=4) as avsp,
            tc.tile_pool(name="rr", bufs=4) as rp,
            tc.tile_pool(name="ys", bufs=4) as ysp,
            tc.tile_pool(name="sps", bufs=2, space="PSUM") as sps,
            tc.tile_pool(name="stp", bufs=2, space="PSUM") as stp,
            tc.tile_pool(name="avp", bufs=2, space="PSUM") as avp,
        ):
            # ---- constants -------------------------------------------------
            wq_t = cst.tile([128, 8, 128], MD)
            wk_t = cst.tile([128, 8, 128], MD)
            wv_t = cst.tile([128, 8, 128], MD)
            for k in range(8):
                nc.sync.dma_start(out=wq_t[:, k, :], in_=wq[k * 128 : (k + 1) * 128, :])
                nc.sync.dma_start(out=wk_t[:, k, :], in_=wk[k * 128 : (k + 1) * 128, :])
                nc.sync.dma_start(out=wv_t[:, k, :], in_=wv[k * 128 : (k + 1) * 128, :])
            wo_t = cst.tile([128, C], MD)
            nc.sync.dma_start(out=wo_t, in_=wo[:, :])
            cos_t = cst.tile([128, T], MD)
            nc.sync.dma_start(out=cos_t, in_=cosd[:, :])
            sin_t = cst.tile([128, T], MD)
            nc.sync.dma_start(out=sin_t, in_=sind[:, :])
            p2_t = cst.tile([128, 128], MD)
            nc.sync.dma_start(out=p2_t, in_=p2d[:, :])
            tri_t = cst.tile([128, 256], MD)  # [tri01 | tri01] for head pairs
            nc.sync.dma_start(out=tri_t, in_=trid[:, :])
            id_t = cst.tile([128, 64], F)
            nc.sync.dma_start(out=id_t, in_=id2d[:, :])

            # persistent double-buffered V^T storage: per head-chunk 256 cols
            # [d 0..63 | ones | zeros*63 | d2 | ones | zeros*63]; the ones
            # column rides the AV matmul to produce softmax denominators and
            # the zeros pad the lhsT to 128 (FWL). zeros/ones are written once.
            VBUF = []
            for vi in range(2):
                Vb = cst.tile([128, 16, 256], MD, name=f"Vb{vi}", tag=f"vb{vi}")
                nc.vector.memset(Vb[:, :, :], 0.0)
                nc.vector.memset(Vb[:, :, 64:256:128], 1.0)
                VBUF.append(Vb)

            QK = {}  # b -> (Qb, Kb)

            # ---- slotted deferred-work chain --------------------------------
            gstep = [0]
            chain_q = []  # (due_step, fn)

            def sched(delay, fn):
                chain_q.append((gstep[0] + delay, fn))

            def drain_chain():
                i = 0
                while i < len(chain_q):
                    due, fn = chain_q[i]
                    if due <= gstep[0]:
                        chain_q.pop(i)
                        fn()
                    else:
                        i += 1

            # ---- phase A: projections + rope + V^T, as pumpable units ------
            def make_a_units(b):
                Qb = qp.tile([128, T], MD, name="Qb")
                Kb = kp.tile([128, T], MD, name="Kb")
                QK[b] = (Qb, Kb)
                Vb = VBUF[b % 2]
                xts = {}
                holders = {}

                def xt_load(nb):
                    def f():
                        xt = xtp.tile([128, 8, 512], MD, name="xt")
                        xts[nb] = xt
                        g0 = b * T + nb * 512
                        for k in range(8):
                            nc.sync.dma_start(
                                out=xt[:, k, :],
                                in_=xT[k * 128 : (k + 1) * 128, g0 : g0 + 512],
                            )
                    return f

                def proj(W_t, key, nb, half):
                    def f():
                        if half == 0:
                            holders[key] = sps.tile([128, 512], F, tag="s", name="ps")
                        ps = holders[key]
                        xt = xts[nb]
                        for k in range(4 * half, 4 * half + 4):
                            nc.tensor.matmul(
                                ps[:, :], lhsT=W_t[:, k, :], rhs=xt[:, k, :],
                                start=(k == 0), stop=(k == 7),
                                skip_group_check=True,
                            )
                    return f

                def rope_a(key, nb):
                    # evacuate + elementwise half of rope; frees the ps bank
                    def f():
                        ps = holders[key]
                        qr = tmp.tile([128, 512], MD, name="qr")
                        nc.scalar.copy(qr[:, :], ps[:, :])
                        qs = tmp.tile([128, 512], MD, name="qs")
                        nc.vector.tensor_tensor(
                            qs[:, :], qr[:, :], sin_t[:, nb * 512 : (nb + 1) * 512],
                            MULT)
                        holders[key] = (qr, qs)
                    return f

                def rope_b(key, nb, dstb):
                    def f():
                        qr, qs = holders.pop(key)
                        cols = slice(nb * 512, (nb + 1) * 512)
                        nc.vector.tensor_tensor(dstb[:, cols], qr[:, :],
                                                cos_t[:, cols], MULT)
                        rot = sps.tile([128, 512], F, tag="s", name="rot")
                        for h in (0, 1):
                            hs = slice(64 * h, 64 * h + 64)
                            nc.tensor.matmul(
                                rot[hs, :], lhsT=p2_t[hs, 64 * h : 64 * h + 64],
                                rhs=qs[hs, :], start=True, stop=True,
                            )
                        nc.vector.tensor_tensor(dstb[:, cols], dstb[:, cols],
                                                rot[:, :], SUB)
                    return f

                def vst_unit(nb):
                    def f():
                        ps = holders["v"]
                        vst = vstp.tile([128, 512], F, name="vst")
                        holders["vst"] = vst
                        nc.scalar.copy(vst[:, :], ps[:, :])
                    return f

                def vtrans(nb, tl):
                    def f():
                        vst = holders["vst"]
                        tt = nb * 4 + tl
                        tcs = slice(tl * 128, (tl + 1) * 128)
                        for h in (0, 1):
                            tp = sps.tile([128, 64], F, tag="s", name="tp")
                            nc.tensor.transpose(
                                tp[:, :], vst[64 * h : 64 * h + 64, tcs],
                                id_t[64 * h : 64 * h + 64, :],
                            )
                            nc.vector.tensor_copy(
                                Vb[:, tt, 128 * h : 128 * h + 64], tp[:, :])
                    return f

                xt_load(0)()  # eager: max DMA lead for the first chunk
                units = []
                for nb in range(4):
                    units.append(proj(wq_t, "q", nb, 0))
                    units.append(proj(wq_t, "q", nb, 1))
                    units.append(rope_a("q", nb))
                    if nb + 1 < 4:
                        units.append(xt_load(nb + 1))
                    units.append(rope_b("q", nb, Qb))
                    units.append(proj(wk_t, "k", nb, 0))
                    units.append(proj(wk_t, "k", nb, 1))
                    units.append(rope_a("k", nb))
                    units.append(rope_b("k", nb, Kb))
                    units.append(proj(wv_t, "v", nb, 0))
                    units.append(proj(wv_t, "v", nb, 1))
                    units.append(vst_unit(nb))
                    for tl in range(4):
                        units.append(vtrans(nb, tl))
                return units

            # ---- y projection for one 128-token tile ------------------------
            def y_tile(b, Ob, i, tl):
                def f():
                    tt = 4 * i + tl
                    lhs = Ob[:, tt * 128 : (tt + 1) * 128]
                    ysb = ysp.tile([128, 1024], MD, name="ysb")
                    for nh in (0, 1):
                        yps = sps.tile([128, 512], F, tag="s", name="yps")
                        nc.tensor.matmul(
                            yps[:, :], lhsT=lhs,
                            rhs=wo_t[:, nh * 512 : (nh + 1) * 512],
                            start=True, stop=True,
                        )
                        if nh == 0:
                            nc.vector.tensor_copy(ysb[:, 0:512], yps[:, :])
                        else:
                            nc.scalar.copy(ysb[:, 512:1024], yps[:, :])
                    r0 = b * T + tt * 128
                    nc.sync.dma_start(out=y[r0 : r0 + 128, :], in_=ysb[:, :])
                return f

            # ---- phase D: attention for batch b, pumping `units` ------------
            def phase_d(b, units):
                Qb, Kb = QK[b]
                Vb = VBUF[b % 2]
                Ob = op_.tile([128, T], MD, name="Ob")
                steps_left = [40]

                def pump():
                    gstep[0] += 1
                    steps_left[0] -= 1
                    drain_chain()
                    if units and steps_left[0] > 0:
                        n = -(-len(units) // steps_left[0])
                        for _ in range(min(n, len(units))):
                            units.pop(0)()
                    elif units:
                        while units:
                            units.pop(0)()

                for i in range(4):
                    av = [avp.tile([128, 512], F, tag="av", name="av")
                          for _ in (0, 1)]
                    nch = 4 * i + 4
                    sts = {}
                    As = {}

                    def emit_st(j):
                        delta = j * 128 - i * 512
                        nl = 512 - max(0, delta)
                        off = 512 - nl
                        st = stp.tile([128, 2, 512], F, name="st")
                        for h in (0, 1):
                            hs = slice(64 * h, 64 * h + 64)
                            nc.tensor.matmul(
                                st[:, h, 0:nl],
                                lhsT=Kb[hs, j * 128 : (j + 1) * 128],
                                rhs=Qb[hs, i * 512 + off : (i + 1) * 512],
                                start=True, stop=True,
                            )
                        sts[j] = (st, off, nl)

                    def emit_exp(j):
                        st, off, nl = sts.pop(j)
                        A = ap_.tile([128, 2, 512], MD, name="A")
                        nc.scalar.activation(
                            A[:, :, 0:nl], st[:, :, 0:nl], EXP, scale=SCALE)
                        if j * 128 >= i * 512:  # diagonal straddle: zero the
                            # above-diagonal triangle (first 128 cols) post-exp
                            nc.vector.tensor_tensor(
                                A[:, :, 0:128], A[:, :, 0:128],
                                tri_t[:, :].rearrange("p (a c) -> p a c", a=2),
                                MULT)
                        As[j] = (A, off, nl)

                    def emit_av(j):
                        A, off, nl = As.pop(j)
                        for h in (0, 1):
                            nc.tensor.matmul(
                                av[h][0:128, off:512],
                                lhsT=Vb[:, j, 128 * h : 128 * h + 128],
                                rhs=A[:, h, 0:nl],
                                start=(j == 0), stop=(j == nch - 1),
                                skip_group_check=True,
                            )

                    emit_st(0)
                    if nch > 1:
                        emit_st(1)
                    emit_exp(0)
                    for j in range(nch):
                        if j + 1 < nch:
                            emit_exp(j + 1)
                        if j + 2 < nch:
                            emit_st(j + 2)
                        pump()
                        emit_av(j)

                    # normalize chain for this i-block, slotted so no engine
                    # FIFO parks behind the DRAM reciprocal round trip
                    def norm_a(i=i, av=av):
                        for h in (0, 1):
                            avs = avsp.tile([65, 512], F, name="avs")
                            nc.vector.tensor_copy(avs[:, :], av[h][0:65, :])
                            srt = rp.tile([128, 4], F, name="srt")
                            nc.sync.dma_start(out=srt[:, :], in_=avs[64:65, :])
                            norm_state[(b, i, h)] = [avs, srt, None]

                    def norm_b(i=i):
                        for h in (0, 1):
                            row = b * 8 + i * 2 + h
                            avs, srt, _ = norm_state[(b, i, h)]
                            rt = rp.tile([128, 4], F, name="rt")
                            nc.vector.reciprocal(rt[:, :], srt[:, :])
                            nc.sync.dma_start(
                                out=scr[row : row + 1, :].rearrange(
                                    "r (p c) -> (r p) c", c=4),
                                in_=rt[:, :],
                            )
                            bct = bcp.tile([64, 512], F, name="bct")
                            src = scr[row : row + 1, :]
                            bap = bass.AP(
                                tensor=src.tensor, offset=src.offset,
                                ap=[[0, 64]] + [list(p) for p in src.ap[1:]],
                            )
                            nc.sync.dma_start(out=bct[:, :], in_=bap)
                            norm_state[(b, i, h)][2] = bct

                    def norm_c(i=i, Ob=Ob):
                        for h in (0, 1):
                            avs, srt, bct = norm_state.pop((b, i, h))
                            nc.vector.tensor_tensor(
                                Ob[64 * h : 64 * h + 64,
                                   i * 512 : (i + 1) * 512],
                                avs[0:64, :], bct[:, :], MULT,
                            )

                    sched(1, norm_a)
                    sched(3, norm_b)
                    sched(5, norm_c)
                    for tl in range(4):
                        sched(6 + tl, y_tile(b, Ob, i, tl))

            norm_state = {}

            # ---- top-level schedule ----------------------------------------
            for u in make_a_units(0):
                u()
            for b in range(B):
                units = make_a_units(b + 1) if b + 1 < B else []
                phase_d(b, units)
            while chain_q:
                gstep[0] += 1
                drain_chain()

    _split_sem_waits(nc)
    return nc


# --------------------------------------------------------------------------
def _host_inputs(x, Wq, Wk, Wv):
    """Per-core input dicts (all shared arrays built once)."""
    BF = np.float16
    xT = np.ascontiguousarray(
        np.asarray(x, dtype=np.float32).reshape(BT, C).T).astype(BF)

    # NeoX d-permutation within each head: evens then odds
    dperm = np.concatenate([np.arange(0, D, 2), np.arange(1, D, 2)])

    inv_freq = (1.0 / (10000.0 ** (np.arange(0, D, 2) / D))).astype(np.float64)
    pos = np.arange(T, dtype=np.float64)
    ang = pos[None, :] * inv_freq[:, None]  # (32, T)
    cos32 = np.cos(ang).astype(np.float32)
    sin32 = np.sin(ang).astype(np.float32)
    cos_t = np.tile(np.vstack([cos32, cos32]), (2, 1))  # (128, T)
    sin_t = np.tile(np.vstack([-sin32, sin32]), (2, 1))  # (128, T), sign folded

    p2 = np.zeros((128, 128), dtype=np.float32)
    for hb in (0, 64):
        for i2 in range(32):
            p2[hb + i2, hb + 32 + i2] = 1.0
            p2[hb + 32 + i2, hb + i2] = 1.0

    tri = np.where(
        np.arange(128)[None, :] >= np.arange(128)[:, None], 1.0, 0.0
    ).astype(np.float32)
    tri2x = np.concatenate([tri, tri], axis=1)  # (128, 256)
    id2 = np.tile(np.eye(D, dtype=np.float32), (2, 1))  # (128, 64)

    Wq = np.asarray(Wq, dtype=np.float32)
    Wk = np.asarray(Wk, dtype=np.float32)
    Wv = np.asarray(Wv, dtype=np.float32)

    in_maps = []
    for c in range(N_CORES):
        sl = slice(128 * c, 128 * (c + 1))
        wq_c = Wq[:, sl].reshape(C, 2, D)[:, :, dperm].reshape(C, 128)
        wk_c = Wk[:, sl].reshape(C, 2, D)[:, :, dperm].reshape(C, 128)
        in_maps.append({
            "xT": xT,
            "wq": np.ascontiguousarray(wq_c).astype(BF),
            "wk": np.ascontiguousarray(wk_c).astype(BF),
            "wv": np.ascontiguousarray(Wv[:, sl]).astype(BF),
            "wo": None,  # set below
            "cos": cos_t.astype(BF),
            "sin2": sin_t.astype(BF),
            "p2": p2.astype(BF),
            "tri2x": tri2x.astype(BF),
            "id2": id2,
        })
    return in_maps


def kernel(x, Wq, Wk, Wv, Wo, bo):
    global _BUILT, LAST_RESULT
    from concourse.bass_utils import run_bass_kernel_spmd

    if TRACE:
        _install_ntff_hook()

    if _BUILT is None:
        _BUILT = _build()
    nc = _BUILT

    in_maps = _host_inputs(x, Wq, Wk, Wv)
    Wo = np.asarray(Wo, dtype=np.float32)
    for c in range(N_CORES):
        in_maps[c]["wo"] = np.ascontiguousarray(
            Wo[128 * c : 128 * (c + 1), :]).astype(np.float16)

    last_err = None
    for attempt in range(3):
        try:
            res = run_bass_kernel_spmd(
                nc, in_maps, core_ids=list(range(N_CORES)), trace=TRACE
            )
            break
        except Exception as e:  # transient NRT device errors: retry
            last_err = e
            import time as _time

            _time.sleep(2.0)
    else:
        raise last_err
    LAST_RESULT = res

    acc = res.results[0]["y"].astype(np.float32)
    for c in range(1, N_CORES):
        acc = acc + res.results[c]["y"].astype(np.float32)
    out = acc + np.asarray(bo, dtype=np.float32)[None, :]
    return out.reshape(B, T, C)



# revision 2
# speedup vs baseline: 1.0560x; 1.0560x over previous
"""Causal multi-head self-attention (RoPE) Trainium2 Bass kernel (v3).

Problem: x:(4,2048,1024), Wq/Wk/Wv:(1024,1024), Wo:(1024,1024), bo:(1024,)
  q,k,v = split_heads(x@W*), rope(q), rope(k), causal softmax(q k^T/8) v, @Wo+bo

Sharding: head-parallel across 8 cores. Core c owns heads {2c, 2c+1} for all
4 batches: it computes q/k/v projections against the 128-column weight slice,
attention for its heads, and a partial output projection against the matching
128-row slice of Wo. Host sums the 8 partial (8192,1024) fp16 outputs and
adds bo.

v3 changes over v2 (v2 measured 384us, PE 78.6% busy, all DMA on one queue):
  - DMA spread over three rings: bulk xT loads ride the gpsimd SWDGE queue
    (idle engine, FIFO blocking is harmless there), y stores + the normalize
    round trips stay on qSP-HWDGE, constants prologue on qACT-HWDGE. xT and
    weight loads are single fused descriptors (strided AP) instead of 8.
  - V is projected token-major (lhsT = x^T chunk, rhs = Wv slice) straight
    into the AV-stationary layout, eliminating 128 PE transposes per core
    and their PSUM evacuation copies.
  - ~2.6us of warmup matmuls at the top of the stream flip the PE HAM clock
    gate to 8/8 while the first DMAs are still in flight.
  - PSUM evacuation copies rebalanced: ACT keeps exp + q/k rope evacuations,
    DVE takes everything else.

On-core layout (all "T" tensors feature-major: partitions=feature, free=tok):
  Q^T/K^T (128 x 2048/batch): rows = [h0 d-evens(32), h0 d-odds(32), h1 ...]
  RoPE: dst = qr*cos - P2@(qr*sin2); P2 is block-diagonal per head, so the
    rot matmul runs as two concurrent 64x64 col-tiled matmuls.
  S^T tiles (tj x ti) = K^T.T @ Q^T per head (64-contraction, the two heads
    run concurrently via row tiling). A = exp(0.125*S^T) in fp16 (max logit
    ~3.6 -> exp <= ~40, far from fp16 overflow); diagonal straddle tiles are
    masked AFTER exp by a 0/1 fp16 triangle multiply (2x DVE mode).
  V (tok-major, per 128-token tile): [d 0..63 | ones | zeros*63] per head;
    the ones column rides the AV matmul to produce softmax denominators and
    the zeros pad the lhsT to 128 (FWL). zeros/ones are written once.
  O~^T (65 x ti) accumulated = [V|1].T @ A over tj chunks; row 64 = softmax
    denominators. Normalize via DRAM-staged reciprocal broadcast + DVE mult.
  y partial (128t x 1024) = O^T-chunk.T @ Wo-slice, fp16, psum->sbuf->DRAM.
"""

import numpy as np

B, T, C = 4, 2048, 1024
H, D = 16, 64
N_CORES = 8
BT = B * T
SCALE = 0.125  # D**-0.5

TRACE = False            # set True (e.g. from test.py) to capture an NTFF trace
LAST_RESULT = None       # BassKernelResults of the most recent run

_BUILT = None            # cached nc


# --------------------------------------------------------------------------
# workaround: this walrus build rejects >1 semaphore wait per instruction
def _split_sem_waits(nc, max_waits=1):
    import concourse.mybir as mybir

    n = 0
    for f in nc.m.functions:
        for bb in f.blocks:
            insts = bb.instructions
            idx = 0
            while idx < len(insts):
                i = insts[idx]
                si = getattr(i, "sync_info", None)
                if si is not None and si.on_wait and len(si.on_wait) > max_waits:
                    waits = list(si.on_wait)
                    extra, keep = waits[:-max_waits], waits[-max_waits:]
                    si.on_wait = keep
                    pos = idx
                    for j in range(0, len(extra), max_waits):
                        n += 1
                        nd = mybir.InstNoOp(name=f"I-waitsplit-{n}", ins=[], outs=[])
                        nd.engine = i.engine
                        nd.sync_info = mybir.SyncInfo(
                            on_wait=extra[j : j + max_waits], on_update=[]
                        )
                        insts.insert(pos, nd)
                        pos += 1
                    idx = pos
                idx += 1


def _install_ntff_hook():
    """The image's antenv lacks axon_hooks; synthesize it so trace=True works."""
    import sys
    import types

    if "antenv.axon_hooks" in sys.modules:
        return
    import antenv

    state = {"hook": None}
    mod = types.ModuleType("antenv.axon_hooks")
    mod.get_axon_ntff_profile_hook = lambda: state["hook"]
    mod.set_axon_ntff_profile_hook = lambda h: state.__setitem__("hook", h)
    sys.modules["antenv.axon_hooks"] = mod
    antenv.axon_hooks = mod
    try:
        import contextlib

        from trn_agent_boot.trn_boot import _ntff_profile_via_ctypes

        inner = _ntff_profile_via_ctypes("/opt/axon/libaxon_pjrt.so")

        # axon_start_nrt_profile needs the PJRT client fully initialized;
        # retry with a forced execute between attempts, degrade to
        # no-profile rather than crash the whole run
        @contextlib.contextmanager
        def robust_hook(output_dir, device_ids):
            import jax.numpy as jnp

            ctx = None
            for attempt in range(3):
                try:
                    jnp.zeros(8).block_until_ready()
                    c = inner(output_dir, device_ids)
                    c.__enter__()
                    ctx = c
                    break
                except Exception as e:  # profile start failed; retry
                    print(f"NTFF start attempt {attempt} failed: {e}")
                    import time as _t

                    _t.sleep(1.0)
            if ctx is None:
                print("NTFF profile unavailable; running without trace")
            try:
                yield
            finally:
                if ctx is not None:
                    ctx.__exit__(None, None, None)

        state["hook"] = robust_hook if inner is not None else None
    except Exception:
        state["hook"] = None


# --------------------------------------------------------------------------
def _build():
    import concourse.bass as bass
    import concourse.mybir as mybir
    from concourse.tile import TileContext

    F = mybir.dt.float32
    MD = mybir.dt.float16  # matmul operand dtype
    MULT = mybir.AluOpType.mult
    SUB = mybir.AluOpType.subtract
    EXP = mybir.ActivationFunctionType.Exp

    nc = bass.Bass()

    xT = nc.dram_tensor("xT", (C, BT), MD, kind="ExternalInput")
    wq = nc.dram_tensor("wq", (C, 128), MD, kind="ExternalInput")
    wk = nc.dram_tensor("wk", (C, 128), MD, kind="ExternalInput")
    wv = nc.dram_tensor("wv", (C, 128), MD, kind="ExternalInput")
    wo = nc.dram_tensor("wo", (128, C), MD, kind="ExternalInput")
    cosd = nc.dram_tensor("cos", (128, T), MD, kind="ExternalInput")
    sind = nc.dram_tensor("sin2", (128, T), MD, kind="ExternalInput")
    p2d = nc.dram_tensor("p2", (128, 128), MD, kind="ExternalInput")
    trid = nc.dram_tensor("tri2x", (128, 256), MD, kind="ExternalInput")
    y = nc.dram_tensor("y", (BT, C), MD, kind="ExternalOutput")
    scr = nc.dram_tensor("scr", (B * 8, 512), F, kind="Internal")

    with TileContext(nc) as tc:
        with (
            tc.tile_pool(name="const", bufs=1) as cst,
            tc.tile_pool(name="xt", bufs=3) as xtp,
            tc.tile_pool(name="qt", bufs=2) as qp,
            tc.tile_pool(name="kt", bufs=2) as kp,
            tc.tile_pool(name="ot", bufs=2) as op_,
            tc.tile_pool(name="tmp", bufs=4) as tmp,
            tc.tile_pool(name="at", bufs=6) as ap_,
            tc.tile_pool(name="bc", bufs=4) as bcp,
            tc.tile_pool(name="avs", bufs=4) as avsp,
            tc.tile_pool(name="rr", bufs=4) as rp,
            tc.tile_pool(name="ys", bufs=4) as ysp,
            tc.tile_pool(name="sps", bufs=2, space="PSUM") as sps,
            tc.tile_pool(name="stp", bufs=2, space="PSUM") as stp,
            tc.tile_pool(name="avp", bufs=2, space="PSUM") as avp,
        ):
            # ---- PE warmup: flip the HAM clock gate while DMAs land --------
            wup = cst.tile([128, 128], MD)
            nc.vector.memset(wup[:, :], 0.0)
            wps = sps.tile([128, 512], F, tag="s", name="wps")
            for _ in range(22):
                nc.tensor.matmul(
                    wps[:, 0:128], lhsT=wup[:, :], rhs=wup[:, :],
                    start=True, stop=True, skip_group_check=True,
                )

            # ---- constants (fused single-descriptor loads) -----------------
            # q/k/v weights + rope tables on the ACT HWDGE ring; the rest on
            # the SP ring so the prologue loads run on two queues in parallel.
            wq_t = cst.tile([128, 8, 128], MD)
            wk_t = cst.tile([128, 8, 128], MD)
            wv_t = cst.tile([128, 8, 128], MD)
            nc.scalar.dma_start(
                out=wq_t[:, :, :],
                in_=wq[:, :].rearrange("(k p) c -> p k c", p=128))
            nc.scalar.dma_start(
                out=wk_t[:, :, :],
                in_=wk[:, :].rearrange("(k p) c -> p k c", p=128))
            nc.scalar.dma_start(
                out=wv_t[:, :, :],
                in_=wv[:, :].rearrange("(k p) c -> p k c", p=128))
            cos_t = cst.tile([128, T], MD)
            nc.scalar.dma_start(out=cos_t, in_=cosd[:, :])
            sin_t = cst.tile([128, T], MD)
            nc.scalar.dma_start(out=sin_t, in_=sind[:, :])
            wo_t = cst.tile([128, C], MD)
            nc.sync.dma_start(out=wo_t, in_=wo[:, :])
            p2_t = cst.tile([128, 128], MD)
            nc.sync.dma_start(out=p2_t, in_=p2d[:, :])
            tri_t = cst.tile([128, 256], MD)  # [tri01 | tri01] for head pairs
            nc.sync.dma_start(out=tri_t, in_=trid[:, :])

            # persistent double-buffered token-major V storage: per 128-token
            # tile 256 cols [d 0..63 | ones | zeros*63 | d2 | ones | zeros*63];
            # the ones column rides the AV matmul to produce softmax
            # denominators and the zeros pad the lhsT to 128 (FWL).
            VBUF = []
            for vi in range(2):
                Vb = cst.tile([128, 16, 256], MD, name=f"Vb{vi}", tag=f"vb{vi}")
                nc.vector.memset(Vb[:, :, :], 0.0)
                nc.vector.memset(Vb[:, :, 64:256:128], 1.0)
                VBUF.append(Vb)

            QK = {}  # b -> (Qb, Kb)

            # ---- slotted deferred-work chain --------------------------------
            gstep = [0]
            chain_q = []  # (due_step, fn)

            def sched(delay, fn):
                chain_q.append((gstep[0] + delay, fn))

            def drain_chain():
                i = 0
                while i < len(chain_q):
                    due, fn = chain_q[i]
                    if due <= gstep[0]:
                        chain_q.pop(i)
                        fn()
                    else:
                        i += 1

            # ---- phase A: projections + rope + V, as pumpable units --------
            def make_a_units(b):
                Qb = qp.tile([128, T], MD, name="Qb")
                Kb = kp.tile([128, T], MD, name="Kb")
                QK[b] = (Qb, Kb)
                Vb = VBUF[b % 2]
                xts = {}
                holders = {}

                def xt_load(nb):
                    def f():
                        xt = xtp.tile([128, 8, 512], MD, name="xt")
                        xts[nb] = xt
                        g0 = b * T + nb * 512
                        nc.gpsimd.dma_start(
                            out=xt[:, :, :],
                            in_=xT[:, g0 : g0 + 512].rearrange(
                                "(k p) t -> p k t", p=128),
                        )
                    return f

                def proj(W_t, key, nb, half):
                    def f():
                        if half == 0:
                            holders[key] = sps.tile([128, 512], F, tag="s", name="ps")
                        ps = holders[key]
                        xt = xts[nb]
                        for k in range(4 * half, 4 * half + 4):
                            nc.tensor.matmul(
                                ps[:, :], lhsT=W_t[:, k, :], rhs=xt[:, k, :],
                                start=(k == 0), stop=(k == 7),
                                skip_group_check=True,
                            )
                    return f

                def rope_a(key, nb):
                    # evacuate + elementwise half of rope; frees the ps bank
                    def f():
                        ps = holders[key]
                        qr = tmp.tile([128, 512], MD, name="qr")
                        nc.scalar.copy(qr[:, :], ps[:, :])
                        qs = tmp.tile([128, 512], MD, name="qs")
                        nc.vector.tensor_tensor(
                            qs[:, :], qr[:, :], sin_t[:, nb * 512 : (nb + 1) * 512],
                            MULT)
                        holders[key] = (qr, qs)
                    return f

                def rope_b(key, nb, dstb):
                    def f():
                        qr, qs = holders.pop(key)
                        cols = slice(nb * 512, (nb + 1) * 512)
                        nc.vector.tensor_tensor(dstb[:, cols], qr[:, :],
                                                cos_t[:, cols], MULT)
                        rot = sps.tile([128, 512], F, tag="s", name="rot")
                        for h in (0, 1):
                            hs = slice(64 * h, 64 * h + 64)
                            nc.tensor.matmul(
                                rot[hs, :], lhsT=p2_t[hs, 64 * h : 64 * h + 64],
                                rhs=qs[hs, :], start=True, stop=True,
                            )
                        nc.vector.tensor_tensor(dstb[:, cols], dstb[:, cols],
                                                rot[:, :], SUB)
                    return f

                # token-major V projection: out[tok, d] = (x^T chunk).T @ Wv
                # slice, 4 token-tiles of 128 accumulated side by side in one
                # PSUM bank (the first matmul's start=True clears the whole
                # bank; later tiles' k=0 matmuls land on clear bits and
                # overwrite, k>0 accumulate).
                def v_mm(nb, tl):
                    def f():
                        if tl == 0:
                            holders["v"] = sps.tile([128, 512], F, tag="s",
                                                    name="vp")
                        vp = holders["v"]
                        xt = xts[nb]
                        tcs = slice(tl * 128, (tl + 1) * 128)
                        for k in range(8):
                            nc.tensor.matmul(
                                vp[:, tcs], lhsT=xt[:, k, tcs],
                                rhs=wv_t[:, k, :],
                                start=(tl == 0 and k == 0),
                                stop=(tl == 3 and k == 7),
                                skip_group_check=True,
                            )
                    return f

                def v_evac(nb):
                    def f():
                        vp = holders.pop("v")
                        vr = vp.rearrange("p (tl d) -> p tl d", tl=4)
                        for h in (0, 1):
                            nc.vector.tensor_copy(
                                Vb[:, 4 * nb : 4 * nb + 4,
                                   128 * h : 128 * h + 64],
                                vr[:, :, 64 * h : 64 * h + 64])
                    return f

                xt_load(0)()  # eager: max DMA lead for the first chunk
                units = []
                for nb in range(4):
                    units.append(proj(wq_t, "q", nb, 0))
                    units.append(proj(wq_t, "q", nb, 1))
                    units.append(rope_a("q", nb))
                    if nb + 1 < 4:
                        units.append(xt_load(nb + 1))
                    units.append(rope_b("q", nb, Qb))
                    units.append(proj(wk_t, "k", nb, 0))
                    units.append(proj(wk_t, "k", nb, 1))
                    units.append(rope_a("k", nb))
                    units.append(rope_b("k", nb, Kb))
                    for tl in range(4):
                        units.append(v_mm(nb, tl))
                    units.append(v_evac(nb))
                return units

            # ---- y projection for one 128-token tile, split in two slots ---
            def y_half(b, Ob, i, tl, nh, ysb_h):
                def f():
                    tt = 4 * i + tl
                    lhs = Ob[:, tt * 128 : (tt + 1) * 128]
                    if nh == 0:
                        ysb_h["t"] = ysp.tile([128, 1024], MD, name="ysb")
                    ysb = ysb_h["t"]
                    yps = sps.tile([128, 512], F, tag="s", name="yps")
                    nc.tensor.matmul(
                        yps[:, :], lhsT=lhs,
                        rhs=wo_t[:, nh * 512 : (nh + 1) * 512],
                        start=True, stop=True,
                    )
                    nc.vector.tensor_copy(
                        ysb[:, nh * 512 : (nh + 1) * 512], yps[:, :])
                    if nh == 1:
                        r0 = b * T + tt * 128
                        nc.sync.dma_start(out=y[r0 : r0 + 128, :], in_=ysb[:, :])
                return f

            # ---- phase D: attention for batch b, pumping `units` ------------
            def phase_d(b, units):
                Qb, Kb = QK[b]
                Vb = VBUF[b % 2]
                Ob = op_.tile([128, T], MD, name="Ob")
                steps_left = [40]

                def pump():
                    gstep[0] += 1
                    steps_left[0] -= 1
                    drain_chain()
                    if units and steps_left[0] > 0:
                        n = -(-len(units) // steps_left[0])
                        for _ in range(min(n, len(units))):
                            units.pop(0)()
                    elif units:
                        while units:
                            units.pop(0)()

                for i in range(4):
                    av = [avp.tile([128, 512], F, tag="av", name="av")
                          for _ in (0, 1)]
                    nch = 4 * i + 4
                    sts = {}
                    As = {}

                    def emit_st(j):
                        delta = j * 128 - i * 512
                        nl = 512 - max(0, delta)
                        off = 512 - nl
                        st = stp.tile([128, 2, 512], F, name="st")
                        for h in (0, 1):
                            hs = slice(64 * h, 64 * h + 64)
                            nc.tensor.matmul(
                                st[:, h, 0:nl],
                                lhsT=Kb[hs, j * 128 : (j + 1) * 128],
                                rhs=Qb[hs, i * 512 + off : (i + 1) * 512],
                                start=True, stop=True,
                            )
                        sts[j] = (st, off, nl)

                    def emit_exp(j):
                        st, off, nl = sts.pop(j)
                        A = ap_.tile([128, 2, 512], MD, name="A")
                        nc.scalar.activation(
                            A[:, :, 0:nl], st[:, :, 0:nl], EXP, scale=SCALE)
                        if j * 128 >= i * 512:  # diagonal straddle: zero the
                            # above-diagonal triangle (first 128 cols) post-exp
                            nc.vector.tensor_tensor(
                                A[:, :, 0:128], A[:, :, 0:128],
                                tri_t[:, :].rearrange("p (a c) -> p a c", a=2),
                                MULT)
                        As[j] = (A, off, nl)

                    def emit_av(j):
                        A, off, nl = As.pop(j)
                        for h in (0, 1):
                            nc.tensor.matmul(
                                av[h][0:128, off:512],
                                lhsT=Vb[:, j, 128 * h : 128 * h + 128],
                                rhs=A[:, h, 0:nl],
                                start=(j == 0), stop=(j == nch - 1),
                                skip_group_check=True,
                            )

                    emit_st(0)
                    if nch > 1:
                        emit_st(1)
                    emit_exp(0)
                    for j in range(nch):
                        if j + 1 < nch:
                            emit_exp(j + 1)
                        if j + 2 < nch:
                            emit_st(j + 2)
                        pump()
                        emit_av(j)

                    # normalize chain for this i-block, slotted so no engine
                    # FIFO parks behind the DRAM reciprocal round trip
                    def norm_a(i=i, av=av):
                        for h in (0, 1):
                            avs = avsp.tile([65, 512], F, name="avs")
                            nc.vector.tensor_copy(avs[:, :], av[h][0:65, :])
                            srt = rp.tile([128, 4], F, name="srt")
                            nc.sync.dma_start(out=srt[:, :], in_=avs[64:65, :])
                            norm_state[(b, i, h)] = [avs, srt, None]

                    def norm_b(i=i):
                        for h in (0, 1):
                            row = b * 8 + i * 2 + h
                            avs, srt, _ = norm_state[(b, i, h)]
                            rt = rp.tile([128, 4], F, name="rt")
                            nc.vector.reciprocal(rt[:, :], srt[:, :])
                            nc.sync.dma_start(
                                out=scr[row : row + 1, :].rearrange(
                                    "r (p c) -> (r p) c", c=4),
                                in_=rt[:, :],
                            )
                            bct = bcp.tile([64, 512], F, name="bct")
                            src = scr[row : row + 1, :]
                            bap = bass.AP(
                                tensor=src.tensor, offset=src.offset,
                                ap=[[0, 64]] + [list(p) for p in src.ap[1:]],
                            )
                            nc.sync.dma_start(out=bct[:, :], in_=bap)
                            norm_state[(b, i, h)][2] = bct

                    def norm_c(i=i, Ob=Ob):
                        for h in (0, 1):
                            avs, srt, bct = norm_state.pop((b, i, h))
                            nc.vector.tensor_tensor(
                                Ob[64 * h : 64 * h + 64,
                                   i * 512 : (i + 1) * 512],
                                avs[0:64, :], bct[:, :], MULT,
                            )

                    sched(1, norm_a)
                    sched(3, norm_b)
                    sched(5, norm_c)
                    ysb_hs = [{} for _ in range(4)]
                    for tl in range(4):
                        sched(6 + tl, y_half(b, Ob, i, tl, 0, ysb_hs[tl]))
                        sched(7 + tl, y_half(b, Ob, i, tl, 1, ysb_hs[tl]))

            norm_state = {}

            # ---- top-level schedule ----------------------------------------
            for u in make_a_units(0):
                u()
            for b in range(B):
                units = make_a_units(b + 1) if b + 1 < B else []
                phase_d(b, units)
            while chain_q:
                gstep[0] += 1
                drain_chain()

    _split_sem_waits(nc)
    return nc


# --------------------------------------------------------------------------
def _host_inputs(x, Wq, Wk, Wv):
    """Per-core input dicts (all shared arrays built once)."""
    BF = np.float16
    xT = np.ascontiguousarray(
        np.asarray(x, dtype=np.float32).reshape(BT, C).T).astype(BF)

    # NeoX d-permutation within each head: evens then odds
    dperm = np.concatenate([np.arange(0, D, 2), np.arange(1, D, 2)])

    inv_freq = (1.0 / (10000.0 ** (np.arange(0, D, 2) / D))).astype(np.float64)
    pos = np.arange(T, dtype=np.float64)
    ang = pos[None, :] * inv_freq[:, None]  # (32, T)
    cos32 = np.cos(ang).astype(np.float32)
    sin32 = np.sin(ang).astype(np.float32)
    cos_t = np.tile(np.vstack([cos32, cos32]), (2, 1))  # (128, T)
    sin_t = np.tile(np.vstack([-sin32, sin32]), (2, 1))  # (128, T), sign folded

    p2 = np.zeros((128, 128), dtype=np.float32)
    for hb in (0, 64):
        for i2 in range(32):
            p2[hb + i2, hb + 32 + i2] = 1.0
            p2[hb + 32 + i2, hb + i2] = 1.0

    tri = np.where(
        np.arange(128)[None, :] >= np.arange(128)[:, None], 1.0, 0.0
    ).astype(np.float32)
    tri2x = np.concatenate([tri, tri], axis=1)  # (128, 256)

    Wq = np.asarray(Wq, dtype=np.float32)
    Wk = np.asarray(Wk, dtype=np.float32)
    Wv = np.asarray(Wv, dtype=np.float32)

    in_maps = []
    for c in range(N_CORES):
        sl = slice(128 * c, 128 * (c + 1))
        wq_c = Wq[:, sl].reshape(C, 2, D)[:, :, dperm].reshape(C, 128)
        wk_c = Wk[:, sl].reshape(C, 2, D)[:, :, dperm].reshape(C, 128)
        in_maps.append({
            "xT": xT,
            "wq": np.ascontiguousarray(wq_c).astype(BF),
            "wk": np.ascontiguousarray(wk_c).astype(BF),
            "wv": np.ascontiguousarray(Wv[:, sl]).astype(BF),
            "wo": None,  # set below
            "cos": cos_t.astype(BF),
            "sin2": sin_t.astype(BF),
            "p2": p2.astype(BF),
            "tri2x": tri2x.astype(BF),
        })
    return in_maps


def kernel(x, Wq, Wk, Wv, Wo, bo):
    global _BUILT, LAST_RESULT
    from concourse.bass_utils import run_bass_kernel_spmd

    if TRACE:
        _install_ntff_hook()

    if _BUILT is None:
        _BUILT = _build()
    nc = _BUILT

    in_maps = _host_inputs(x, Wq, Wk, Wv)
    Wo = np.asarray(Wo, dtype=np.float32)
    for c in range(N_CORES):
        in_maps[c]["wo"] = np.ascontiguousarray(
            Wo[128 * c : 128 * (c + 1), :]).astype(np.float16)

    last_err = None
    for attempt in range(3):
        try:
            res = run_bass_kernel_spmd(
                nc, in_maps, core_ids=list(range(N_CORES)), trace=TRACE
            )
            break
        except Exception as e:  # transient NRT device errors: retry
            last_err = e
            import time as _time

            _time.sleep(2.0)
    else:
        raise last_err
    LAST_RESULT = res

    acc = res.results[0]["y"].astype(np.float32)
    for c in range(1, N_CORES):
        acc = acc + res.results[c]["y"].astype(np.float32)
    out = acc + np.asarray(bo, dtype=np.float32)[None, :]
    return out.reshape(B, T, C)
